# revision 27
# baseline (speedup 1.0000x reference)
"""Trainium2 Bass kernel for single-head cross-attention.

Reference computation (B=4, Sq=Skv=2048, D=1024, fp32):
    Q = query @ Wq + bq ; K = key @ Wk + bk ; V = value @ Wv + bv
    out = softmax(Q K^T / sqrt(D)) V @ Wo + bo

Single-head attention is a bilinear form, so the host folds the weight
pairs once per call:
    M  = Wq @ Wk^T            scores = query @ M @ key^T (+ bias terms)
    N  = Wv @ Wo              out_unnorm = (A @ value) @ N
which removes the K and V projections (and any cross-core collective)
from the device program entirely. Bias algebra (exact for any biases):
  * bk adds a per-QUERY-row constant to scores -> cancels in softmax.
  * bq adds d_k = key_k . (Wk @ bq) per KV column -> folded into the
    Exp activation's per-partition bias (d/sqrt(D), host-computed).
  * bv adds sums * (bv @ Wo) to the unnormalized output -> folded with
    bo into bo2 = bv @ Wo + bo, added after the 1/sums normalization.

Sharding: 8 shards = (batch b in 0..3) x (query half h in 0..1); core
c = 2*b + h computes output rows [h*1024,(h+1)*1024) of batch b from
its query half plus the full key/value of its batch (replicated reads,
no collectives).

Device dataflow (transpose-free; host ships query/key feature-major):
    X^T[e,q]   = M^T @ qT          (lhsT=M chunks, rhs=qT)
    S^T[kv,q]  = key @ X^T         (lhsT=keyT,     rhs=X^T)
    A^T        = exp(S^T/32 + d/32)            (unnormalized)
    O^T[dv,q]  = value^T @ A^T     (lhsT=value,    rhs=A^T)
    sums[q,1]  = A @ ones          (lhsT=A^T,      rhs=ones)
    F[q,f]     = O @ N             (lhsT=O^T,      rhs=N)
    out        = F * (1/sums) + bo2
"""

import sys

if "/opt/trn_rl_repo" not in sys.path:
    sys.path.insert(0, "/opt/trn_rl_repo")

from contextlib import ExitStack

import ml_dtypes
import numpy as np

import concourse.bass as bass
import concourse.mybir as mybir
import concourse.tile as tile
from concourse import bacc
from concourse.bass_utils import run_bass_kernel_spmd

B, SQ, SKV, D = 4, 2048, 2048, 1024
NCORES = 8
QL = SQ // 2  # local query rows per core
P = 128
DC = D // P  # feature chunks (8)
KVC = SKV // P  # kv chunks (16)
N5 = 512
F32 = mybir.dt.float32
CDT = mybir.dt.bfloat16  # on-device compute dtype for matmul operands
F8 = mybir.dt.float8e4  # scores matmul runs fp8 e4m3 in DoubleRow mode
NP_CDT = ml_dtypes.bfloat16
NP_F8 = ml_dtypes.float8_e4m3fn
SCALE = 1.0 / 32.0  # 1/sqrt(D)
DR = mybir.MatmulPerfMode.DoubleRow
NH = 256  # DoubleRow moving tile: 2 k-chunks x 256 output columns

AF = mybir.ActivationFunctionType


def _build_tile(ctx: ExitStack, tc, aps):
    nc = tc.nc
    qT, keyT, val, m8, n8, dbias, bo2, out = aps

    weights = ctx.enter_context(tc.tile_pool(name="weights", bufs=1))
    big = ctx.enter_context(tc.tile_pool(name="big", bufs=1))
    streams = ctx.enter_context(tc.tile_pool(name="streams", bufs=3))
    evac = ctx.enter_context(tc.tile_pool(name="evac", bufs=4))
    psum = ctx.enter_context(tc.tile_pool(name="psum", bufs=4, space="PSUM"))
    psum_s = ctx.enter_context(tc.tile_pool(name="psum_s", bufs=2, space="PSUM"))

    # All input DMA rides ONE ring (sync) in exactly the order compute
    # consumes it: the 16 underlying DMA engines give a single ring the
    # full bus, and a second ring would only let later, less urgent
    # transfers (key/value/N) steal descriptor slots from the m/q pairs
    # the first matmuls are stalled on. Output DMA gets its own ring; it
    # only flows after the input burst has drained. Each dma_start costs
    # ~0.65us of ring-sequencer issue time and each descriptor ~40ns of
    # DMA-engine overhead, so all inputs arrive host-packed in SBUF
    # layout (4-16 KiB contiguous runs) and are batched into few
    # instructions; the X inputs stream as (m, q) pairs of TWO d-chunks
    # each so the first matmul starts after ~0.8 MiB.
    m_all = weights.tile([P, DC, D], CDT, tag="m")
    q_in0 = streams.tile([P, DC, N5], CDT, tag="xin")
    for dc in range(0, DC, 2):
        nc.sync.dma_start(out=m_all[:, dc : dc + 2, :], in_=m8[:, dc : dc + 2, :])
        nc.sync.dma_start(
            out=q_in0[:, dc : dc + 2, :], in_=qT[:, 0, dc : dc + 2, :]
        )

    # Warm-up: the PE clock ramps to full speed only after ~3us of
    # continuous execution. Dummy matmuls on a memset tile fill the
    # DMA-latency head so the real X matmuls start already ramped.
    junk = weights.tile([P, P], CDT, tag="junk")
    nc.gpsimd.memset(junk, 0.0)
    for _ in range(30):
        ps_w = psum_s.tile([P, P], F32, tag="warm")
        nc.tensor.matmul(ps_w, lhsT=junk, rhs=junk, start=True, stop=True)

    # ---- X^T = M^T @ qT --------------------------------------------------
    # X and key are the fp8 operand pair of the DoubleRow scores matmul.
    xTo = big.tile([P, DC, QL], F8, tag="xTo")  # X^T: [e%128, e//128, q]

    def x_block(x_in, j):
        for ec in range(DC):
            ps = psum.tile([P, N5], F32, tag="mm")
            for dc in range(DC):
                nc.tensor.matmul(
                    ps,
                    lhsT=m_all[:, dc, ec * P : (ec + 1) * P],
                    rhs=x_in[:, dc, :],
                    start=(dc == 0),
                    stop=(dc == DC - 1),
                )
            nc.vector.tensor_copy(out=xTo[:, ec, j * N5 : (j + 1) * N5], in_=ps)

    x_block(q_in0, 0)
    for j in range(1, QL // N5):
        x_in = streams.tile([P, DC, N5], CDT, tag="xin")
        nc.sync.dma_start(out=x_in, in_=qT[:, j, :, :])
        x_block(x_in, j)

    # key/value/N stream behind the X inputs, in consumption order.
    kT_s = big.tile([P, DC, SKV], F8, tag="kT")  # key^T: [e%128, e//128, kv]
    nc.sync.dma_start(out=kT_s, in_=keyT)
    d_s = weights.tile([P, KVC], F32, tag="dbias")
    nc.sync.dma_start(out=d_s, in_=dbias)
    v_s = big.tile([P, KVC, D], CDT, tag="v")  # value: [kv%128, kv//128, dv]
    for c in range(0, KVC, 8):
        nc.sync.dma_start(out=v_s[:, c : c + 8, :], in_=val[:, c : c + 8, :])
    n_all = weights.tile([P, DC, D], CDT, tag="n")
    nc.sync.dma_start(out=n_all, in_=n8)
    bo2_s = weights.tile([P, D], F32, tag="bo2")
    bo2_bcast = bass.AP(tensor=bo2.tensor, offset=bo2.offset, ap=[[0, P], bo2.ap[0]])
    nc.sync.dma_start(out=bo2_s, in_=bo2_bcast)
    ones = weights.tile([P, 1], F32, tag="ones")
    nc.vector.memset(ones, 1.0)

    # ---- attention + output projection, per 512-query block -----------------
    attn_pool = ctx.enter_context(tc.tile_pool(name="attn", bufs=1))
    for qb in range(QL // N5):
        # scores^T -> exp (with per-kv bias d/32 folded into the activation).
        # fp8 DoubleRow: each matmul contracts TWO 128-row e-chunks
        # (lhsT [128,2,128], rhs [128,2,256] -> out [128,256]).
        attnT = attn_pool.tile([P, KVC, N5], CDT, tag="attnT")
        for c in range(KVC):
            ps = psum.tile([P, N5], F32, tag="mm")
            for nh in range(N5 // NH):
                # nh outer: the two psum accumulation groups must not
                # interleave within one bank (start would re-zero)
                col0 = qb * N5 + nh * NH
                for ecp in range(0, DC, 2):
                    nc.tensor.matmul(
                        ps[:, nh * NH : (nh + 1) * NH],
                        lhsT=kT_s[:, ecp : ecp + 2, c * P : (c + 1) * P],
                        rhs=xTo[:, ecp : ecp + 2, col0 : col0 + NH],
                        start=(ecp == 0),
                        stop=(ecp == DC - 2),
                        perf_mode=DR,
                    )
            nc.scalar.activation(
                out=attnT[:, c, :],
                in_=ps,
                func=AF.Exp,
                bias=d_s[:, c : c + 1],
                scale=SCALE,
            )

        # softmax denominators off the PE: DVE-reduce A^T over kv chunks,
        # then one tiny ones-matmul per 128-query block for the partition sum
        red = evac.tile([P, N5], F32, tag="red")
        nc.vector.tensor_tensor(
            out=red, in0=attnT[:, 0, :], in1=attnT[:, 1, :], op=mybir.AluOpType.add
        )
        for c in range(2, KVC):
            nc.vector.tensor_tensor(
                out=red, in0=red, in1=attnT[:, c, :], op=mybir.AluOpType.add
            )
        ps_sum = psum_s.tile([P, N5 // P], F32, tag="sums")
        for s in range(N5 // P):
            nc.tensor.matmul(
                ps_sum[:, s : s + 1],
                lhsT=red[:, s * P : (s + 1) * P],
                rhs=ones[:, :1],
                start=True,
                stop=True,
            )
        r_s = evac.tile([P, N5 // P], F32, tag="recip")
        nc.vector.reciprocal(r_s, ps_sum)

        # O^T[dv, q] = value^T @ A^T
        outT = attn_pool.tile([P, DC, N5], CDT, tag="outT")
        for m in range(DC):
            ps = psum.tile([P, N5], F32, tag="mm")
            for c in range(KVC):
                nc.tensor.matmul(
                    ps,
                    lhsT=v_s[:, c, m * P : (m + 1) * P],
                    rhs=attnT[:, c, :],
                    start=(c == 0),
                    stop=(c == KVC - 1),
                )
            nc.vector.tensor_copy(out=outT[:, m, :], in_=ps)

        # F[q, f] = O @ N ; out = F * (1/sums) + bo2
        last = qb == QL // N5 - 1
        for s in range(N5 // P):
            fin = evac.tile([P, D], F32, tag="fin")
            row0 = qb * N5 + s * P
            for nf in range(D // N5):
                ps = psum.tile([P, N5], F32, tag="mm")
                for m in range(DC):
                    nc.tensor.matmul(
                        ps,
                        lhsT=outT[:, m, s * P : (s + 1) * P],
                        rhs=n_all[:, m, nf * N5 : (nf + 1) * N5],
                        start=(m == 0),
                        stop=(m == DC - 1),
                    )
                nc.vector.scalar_tensor_tensor(
                    out=fin[:, nf * N5 : (nf + 1) * N5],
                    in0=ps,
                    scalar=r_s[:, s : s + 1],
                    in1=bo2_s[:, nf * N5 : (nf + 1) * N5],
                    op0=mybir.AluOpType.mult,
                    op1=mybir.AluOpType.add,
                )
                if last and s == N5 // P - 1:
                    # final block: ship each half as soon as its STT lands
                    # so the last transfer only covers 256 KiB
                    nc.scalar.dma_start(
                        out=out[row0 : row0 + P, nf * N5 : (nf + 1) * N5],
                        in_=fin[:, nf * N5 : (nf + 1) * N5],
                    )
            if not (last and s == N5 // P - 1):
                nc.scalar.dma_start(out=out[row0 : row0 + P, :], in_=fin)


def build_program():
    nc = bacc.Bacc(
        "TRN2", target_bir_lowering=False, debug=False, num_devices=NCORES
    )
    # All inputs host-packed to the SBUF tile layout (partition-major,
    # contiguous per-partition runs) so DMA descriptors are 4-16 KiB.
    qT = nc.dram_tensor("qT", [P, QL // N5, DC, N5], CDT, kind="ExternalInput").ap()
    keyT = nc.dram_tensor("keyT", [P, DC, SKV], F8, kind="ExternalInput").ap()
    val = nc.dram_tensor("val", [P, KVC, D], CDT, kind="ExternalInput").ap()
    m8 = nc.dram_tensor("m8", [P, DC, D], CDT, kind="ExternalInput").ap()
    n8 = nc.dram_tensor("n8", [P, DC, D], CDT, kind="ExternalInput").ap()
    dbias = nc.dram_tensor("dbias", [P, KVC], F32, kind="ExternalInput").ap()
    bo2 = nc.dram_tensor("bo2", [D], F32, kind="ExternalInput").ap()
    out = nc.dram_tensor("out", [QL, D], F32, kind="ExternalOutput").ap()

    with tile.TileContext(nc) as tc:
        with ExitStack() as ctx:
            _build_tile(ctx, tc, (qT, keyT, val, m8, n8, dbias, bo2, out))
    nc.compile()
    return nc


def prep_in_maps(query, key, value, Wq, bq, Wk, bk, Wv, bv, Wo, bo):
    """Host-side shard prep: fold weight pairs, slice, transpose, cast."""
    query = np.asarray(query, np.float32)
    key = np.asarray(key, np.float32)
    value = np.asarray(value, np.float32)
    Wq = np.asarray(Wq, np.float32)
    Wk = np.asarray(Wk, np.float32)
    Wv = np.asarray(Wv, np.float32)
    Wo = np.asarray(Wo, np.float32)
    bq = np.asarray(bq, np.float32)
    def pack(x, np_dt):
        # [R*P, C] row-major -> [P, R, C]: partition-major SBUF tile layout
        r = x.shape[0] // P
        return np.ascontiguousarray(
            x.reshape(r, P, x.shape[1]).transpose(1, 0, 2)
        ).astype(np_dt)

    shared = {
        "m8": pack(Wq @ Wk.T, NP_CDT),
        "n8": pack(Wv @ Wo, NP_CDT),
        "bo2": (np.asarray(bv, np.float32) @ Wo + np.asarray(bo, np.float32)),
    }
    wkbq = Wk @ bq
    in_maps = []
    for b in range(B):
        kTb = pack(key[b].T, NP_F8)  # [P, DC, SKV]
        vb = pack(value[b], NP_CDT)  # [P, KVC, D]
        # per-kv score bias d/32, laid out [kv%128, kv//128] for ACT bias
        db = ((key[b] @ wkbq) * SCALE).reshape(KVC, P).T.copy()
        for h in range(2):
            # [P, j, DC, N5]: q-block-major so each X stream slice is one
            # contiguous (dc-pair x 512) run per partition
            qTb = np.ascontiguousarray(
                query[b, h * QL : (h + 1) * QL]
                .T.reshape(DC, P, QL // N5, N5)
                .transpose(1, 2, 0, 3)
            ).astype(NP_CDT)
            in_maps.append(
                {"qT": qTb, "keyT": kTb, "val": vb, "dbias": db, **shared}
            )
    return in_maps


_NC_CACHE = None


def _get_nc():
    global _NC_CACHE
    if _NC_CACHE is None:
        _NC_CACHE = build_program()
    return _NC_CACHE


def run(inputs, **run_kwargs):
    nc = _get_nc()
    in_maps = prep_in_maps(**inputs)
    res = run_bass_kernel_spmd(nc, in_maps, core_ids=list(range(NCORES)), **run_kwargs)
    out = np.empty((B, SQ, D), np.float32)
    for b in range(B):
        for h in range(2):
            out[b, h * QL : (h + 1) * QL] = res.results[2 * b + h]["out"]
    return out, res


def kernel(query, key, value, Wq, bq, Wk, bk, Wv, bv, Wo, bo):
    out, _ = run(
        dict(
            query=query, key=key, value=value, Wq=Wq, bq=bq, Wk=Wk, bk=bk,
            Wv=Wv, bv=bv, Wo=Wo, bo=bo,
        )
    )
    return out


if __name__ == "__main__":
    rng = np.random.default_rng(0)
    ins = {
        "query": rng.standard_normal((B, SQ, D), dtype=np.float32),
        "key": rng.standard_normal((B, SKV, D), dtype=np.float32),
        "value": rng.standard_normal((B, SKV, D), dtype=np.float32),
        "Wq": (rng.standard_normal((D, D), dtype=np.float32) * 0.02),
        "bq": np.zeros(D, np.float32),
        "Wk": (rng.standard_normal((D, D), dtype=np.float32) * 0.02),
        "bk": np.zeros(D, np.float32),
        "Wv": (rng.standard_normal((D, D), dtype=np.float32) * 0.02),
        "bv": np.zeros(D, np.float32),
        "Wo": (rng.standard_normal((D, D), dtype=np.float32) * 0.02),
        "bo": np.zeros(D, np.float32),
    }
    out = kernel(**ins)
    print("kernel ran, out shape", out.shape)
